# revision 2
# baseline (speedup 1.0000x reference)
"""GIN 2-layer (GINConv + MLP + BN + ReLU) x2 on 8 Trainium2 NeuronCores — v2.

Strategy (vs v1's per-column indirect DMA at ~12 ns/row):
- dst-shard nodes across 8 cores (12500 each, 98 blocks of 128 slots).
- Aggregation per block: batched `dma_gather` (mlp-library SWDGE ucode,
  thousands of int16 indices per instruction) pulls every in-edge's source
  row (bf16) into SBUF in arbitrary order, then a one-hot matmul
  (M[e,dst] = (slot[e]==dst), built by one broadcast is_equal per block)
  accumulates agg[dst,F] on the PE in fp32 PSUM. Self-edges are just extra
  edges. int16 index range forces a 4-way split of the gather table
  (2 core-shards + 1 zero row per group = 25089 rows < 32767); pad entries
  point at the zero row with slot=255 (one-hot row of zeros).
- MLP runs in transposed space (2 PE transposes per block), biases/BN per
  partition. hpre for the whole shard stays in SBUF; BN batch stats are
  summed on-device and AllReduce'd across the 8 cores, so each launch
  finishes its BatchNorm+ReLU itself: 2 launches total, and the only host
  step between them is concatenating the 8 bf16 h1 shards into the shared
  layer-2 gather table.
"""
import sys

sys.path.insert(0, "/opt/trn_rl_repo")

import numpy as np
from concourse import bass, mybir, library_config
import concourse.tile as tile
from concourse.bass_utils import run_bass_kernel_spmd
from concourse.library_overlay import lower_extended_insts
from concourse.masks import make_identity
from neuron_dtypes import bfloat16

N_NODES = 100000
N_CORES = 8
P = 128
PER_CORE = N_NODES // N_CORES          # 12500
BLOCKS = (PER_CORE + P - 1) // P       # 98
SLOTS = BLOCKS * P                     # 12544
LAST_VALID = PER_CORE - (BLOCKS - 1) * P   # 84 valid cols in last block
GROUPS = 4
GROUP_ROWS = 2 * SLOTS + 1             # 25089 (2 shards + zero row)
ZROW_LOCAL = 2 * SLOTS                 # 25088
TABLE_ROWS = GROUPS * GROUP_ROWS       # 100356
F_IN, H1, H2 = 256, 128, 32
BN_EPS = 1e-5
F32 = mybir.dt.float32
BF16 = mybir.dt.bfloat16
I16 = mybir.dt.int16


# ---------------------------------------------------------------- wait split
def _split_sync_waits(nc, max_waits=1):
    """This container's walrus rejects >1 sync wait per instruction; hoist
    extras onto preceding same-engine NoOps."""
    ctr = 0
    for f in nc.m.functions:
        for bb in f.blocks:
            out, changed = [], False
            for inst in list(bb.instructions):
                si = getattr(inst, "sync_info", None)
                if si is not None and si.on_wait and len(si.on_wait) > max_waits:
                    waits = list(si.on_wait)
                    extra, keep = waits[:-max_waits], waits[-max_waits:]
                    for i in range(0, len(extra), max_waits):
                        ctr += 1
                        out.append(mybir.InstNoOp(
                            name=f"waitsplit-nop-{ctr}", ins=[], outs=[],
                            engine=inst.engine,
                            sync_info=mybir.SyncInfo(
                                on_wait=extra[i:i + max_waits], on_update=[]),
                        ))
                    inst.sync_info = mybir.SyncInfo(
                        on_wait=keep, on_update=list(si.on_update or []))
                    changed = True
                out.append(inst)
            if changed:
                bb.instructions = out
    return ctr


# ---------------------------------------------------------------- host plan
def _table_row_of_node(n):
    """Global node id -> row in the 4-group gather table."""
    core = n // PER_CORE
    return (core // 2) * GROUP_ROWS + (core % 2) * SLOTS + (n - core * PER_CORE)


def _balance_perm(deg):
    """Snake-deal dsts (sorted by degree desc) into 98 blocks so per-block
    edge totals are even across blocks (and hence across cores). Returns
    pos_of[local] = slot index in [0, SLOTS); block 97 keeps the 44 pads."""
    order = np.argsort(-deg, kind="stable")
    cap = np.full(BLOCKS, P, dtype=np.int64)
    cap[BLOCKS - 1] = LAST_VALID
    fill = np.zeros(BLOCKS, dtype=np.int64)
    pos_of = np.empty(PER_CORE, dtype=np.int64)
    b, direction = 0, 1
    for n in order:
        tries = 0
        while fill[b] >= cap[b]:
            b += direction
            if b == BLOCKS or b < 0:
                direction = -direction
                b += direction
            tries += 1
            assert tries <= 2 * BLOCKS
        pos_of[n] = b * P + fill[b]
        fill[b] += 1
        b += direction
        if b == BLOCKS or b < 0:
            direction = -direction
            b += direction
    return pos_of


def _build_plan(edge_index):
    # self term (GIN eps=0: h_i includes x_i) is handled by an identity
    # matmul on a plain sequential DMA of the core's own shard, not here.
    src = np.asarray(edge_index[0], dtype=np.int64)
    dst = np.asarray(edge_index[1], dtype=np.int64)

    core_of = dst // PER_CORE
    # pass 1: per-core dst->slot permutation balancing block loads
    pos_of_all = np.empty(N_NODES, dtype=np.int64)  # node -> slot in its core
    for c in range(N_CORES):
        sel = core_of == c
        deg = np.bincount(dst[sel] - c * PER_CORE, minlength=PER_CORE)
        pos_of_all[c * PER_CORE:(c + 1) * PER_CORE] = _balance_perm(deg)

    score = src // PER_CORE
    g_of = score // 2
    srow = (score % 2) * SLOTS + pos_of_all[src]      # local row in group

    counts = np.zeros((N_CORES, BLOCKS, GROUPS), dtype=np.int64)
    per_core = []
    for c in range(N_CORES):
        sel = np.nonzero(core_of == c)[0]
        spos = pos_of_all[dst[sel]]
        b = spos // P
        slot = spos % P
        g = g_of[sel]
        sr = srow[sel]
        order = np.lexsort((sr, g, b))
        b, slot, g, sr = b[order], slot[order], g[order], sr[order]
        key = b * GROUPS + g
        counts[c] = np.bincount(key, minlength=BLOCKS * GROUPS).reshape(
            BLOCKS, GROUPS)
        per_core.append((key, slot, sr))

    C_bg = (counts.max(axis=0) + P - 1) // P          # [BLOCKS, GROUPS] chunks
    C_b = C_bg.sum(axis=1)                            # cols per block
    TOTC = int(C_b.sum())
    off_flat = np.zeros(BLOCKS * GROUPS + 1, dtype=np.int64)
    off_flat[1:] = np.cumsum(C_bg.reshape(-1))        # chunk-col offset per (b,g)
    off_b = np.zeros(BLOCKS, dtype=np.int64)
    off_b[1:] = np.cumsum(C_b)[:-1]

    cores = []
    for c in range(N_CORES):
        key, slot, sr = per_core[c]
        first = np.searchsorted(key, np.arange(BLOCKS * GROUPS), side="left")
        rank = np.arange(len(key)) - first[key]
        pos = off_flat[key] * P + rank                # global edge position
        gidx16 = np.full((16, TOTC * 8), ZROW_LOCAL, dtype=np.int16)
        gidx16[pos % 16, pos // 16] = sr.astype(np.int16)
        slotf = np.full((P, TOTC), 255.0, dtype=np.float32)
        slotf[pos % P, pos // P] = slot.astype(np.float32)
        cores.append(dict(gidx=np.tile(gidx16, (8, 1)),
                          slotf=slotf.astype(bfloat16)))

    return dict(C_bg=C_bg, C_b=C_b, TOTC=TOTC, off_b=off_b, cores=cores,
                pos_of_all=pos_of_all)


# ---------------------------------------------------------------- BN coeffs
def _emit_bn_coeffs(nc, constp, psump, dramp, s_sum, s_sq, g_t, be_t, nf,
                    n_count):
    """AllReduce raw sums across cores, then per-partition scale/shift."""
    st = constp.tile([nf, 2], F32, tag="bn_st")
    nc.vector.tensor_copy(out=st[:, 0:1], in_=s_sum[:])
    nc.vector.tensor_copy(out=st[:, 1:2], in_=s_sq[:])
    bi = dramp.tile([nf, 2], F32)
    bo = dramp.tile([nf, 2], F32)
    nc.gpsimd.dma_start(bi[:], st[:])
    nc.gpsimd.collective_compute(
        "AllReduce", mybir.AluOpType.add,
        replica_groups=[list(range(N_CORES))],
        ins=[bi.opt()], outs=[bo.opt()])
    st2 = constp.tile([nf, 2], F32, tag="bn_st2")
    nc.gpsimd.dma_start(st2[:], bo[:])

    mean = constp.tile([nf, 1], F32, tag="bn_mean")
    nc.scalar.mul(mean[:], st2[:, 0:1], 1.0 / n_count)
    ex2 = constp.tile([nf, 1], F32, tag="bn_ex2")
    nc.scalar.mul(ex2[:], st2[:, 1:2], 1.0 / n_count)
    msq = constp.tile([nf, 1], F32, tag="bn_msq")
    nc.vector.tensor_tensor(out=msq[:], in0=mean[:], in1=mean[:],
                            op=mybir.AluOpType.mult)
    var = constp.tile([nf, 1], F32, tag="bn_var")
    nc.vector.tensor_tensor(out=var[:], in0=ex2[:], in1=msq[:],
                            op=mybir.AluOpType.subtract)
    veps = constp.tile([nf, 1], F32, tag="bn_veps")
    nc.vector.tensor_scalar_add(veps[:], var[:], BN_EPS)
    std = constp.tile([nf, 1], F32, tag="bn_std")
    nc.scalar.sqrt(std[:], veps[:])
    istd = constp.tile([nf, 1], F32, tag="bn_istd")
    nc.vector.reciprocal(istd[:], std[:])
    scale = constp.tile([nf, 1], F32, tag="bn_scale")
    nc.vector.tensor_tensor(out=scale[:], in0=g_t[:], in1=istd[:],
                            op=mybir.AluOpType.mult)
    mscale = constp.tile([nf, 1], F32, tag="bn_mscale")
    nc.vector.tensor_tensor(out=mscale[:], in0=mean[:], in1=scale[:],
                            op=mybir.AluOpType.mult)
    shift = constp.tile([nf, 1], F32, tag="bn_shift")
    nc.vector.tensor_tensor(out=shift[:], in0=be_t[:], in1=mscale[:],
                            op=mybir.AluOpType.subtract)
    return scale, shift


# ---------------------------------------------------------------- launches
def _build_launch(plan, layer):
    """layer 1: x(bf16,F_IN) -> h1relu shard (bf16, node-major).
    layer 2: h1relu table -> final output shard (f32, node-major)."""
    fin = F_IN if layer == 1 else H1
    fout = H1 if layer == 1 else H2
    out_dt = BF16 if layer == 1 else F32
    C_bg, C_b, off_b, TOTC = (plan["C_bg"], plan["C_b"], plan["off_b"],
                              plan["TOTC"])

    nc = bass.Bass("TRN2", target_bir_lowering=False, debug=False,
                   num_devices=N_CORES, num_swdge_queues=4)
    tab = nc.dram_tensor("tab", [TABLE_ROWS, fin], BF16, kind="ExternalInput")
    gidx = nc.dram_tensor("gidx", [P, TOTC * 8], I16, kind="ExternalInput")
    slotf = nc.dram_tensor("slotf", [P, TOTC], BF16, kind="ExternalInput")
    wa = nc.dram_tensor("wa", [fin, fout], BF16, kind="ExternalInput")
    ba = nc.dram_tensor("ba", [fout, 1], F32, kind="ExternalInput")
    wb = nc.dram_tensor("wb", [fout, fout], BF16, kind="ExternalInput")
    bb = nc.dram_tensor("bb", [fout, 1], F32, kind="ExternalInput")
    gg = nc.dram_tensor("gg", [fout, 1], F32, kind="ExternalInput")
    be = nc.dram_tensor("be", [fout, 1], F32, kind="ExternalInput")
    xself = nc.dram_tensor("xself", [SLOTS, fin], BF16, kind="ExternalInput")
    ho = nc.dram_tensor("ho", [SLOTS, fout], out_dt, kind="ExternalOutput")

    kchunks = fin // P  # K chunks for the first MLP matmul (2 or 1)
    with tile.TileContext(nc) as tc:
        with (
            tc.tile_pool(name="const", bufs=1) as constp,
            tc.tile_pool(name="idxp", bufs=3) as idxp,
            tc.tile_pool(name="gat", bufs=3) as gatp,
            tc.tile_pool(name="mp", bufs=3) as mp,
            tc.tile_pool(name="work", bufs=4) as workp,
            tc.tile_pool(name="big", bufs=1) as bigp,
            tc.tile_pool(name="psum", bufs=1, space="PSUM") as psump,
            tc.tile_pool(name="dram", bufs=2, space="DRAM") as dramp,
        ):
            # InstIota (and make_identity's affine_select) live in the
            # standard Q7 library — emit them BEFORE switching to mlp.
            iota = constp.tile([P, P], BF16)
            nc.gpsimd.iota(iota[:], pattern=[[1, P]], base=0,
                           channel_multiplier=0,
                           allow_small_or_imprecise_dtypes=True)
            ident = constp.tile([P, P], BF16)
            make_identity(nc, ident[:])
            nc.gpsimd.load_library(library_config.mlp)
            wa_t = [constp.tile([P, fout], BF16, tag=f"wa{h}", name=f"wa{h}")
                    for h in range(kchunks)]
            for h in range(kchunks):
                nc.sync.dma_start(out=wa_t[h][:], in_=wa[h * P:(h + 1) * P, :])
            wb_t = constp.tile([fout, fout], BF16)
            nc.sync.dma_start(out=wb_t[:], in_=wb[:, :])
            ba_t = constp.tile([fout, 1], F32)
            nc.sync.dma_start(out=ba_t[:], in_=ba[:, :])
            bb_t = constp.tile([fout, 1], F32)
            nc.sync.dma_start(out=bb_t[:], in_=bb[:, :])
            g_t = constp.tile([fout, 1], F32)
            nc.sync.dma_start(out=g_t[:], in_=gg[:, :])
            be_t = constp.tile([fout, 1], F32)
            nc.sync.dma_start(out=be_t[:], in_=be[:, :])
            slot_all = constp.tile([P, TOTC], BF16)
            nc.sync.dma_start(out=slot_all[:], in_=slotf[:, :])
            mask = constp.tile([fout, P], F32)
            nc.vector.memset(mask[:], 1.0)
            nc.vector.memset(mask[:, LAST_VALID:], 0.0)
            s_sum = constp.tile([fout, 1], F32)
            s_sq = constp.tile([fout, 1], F32)
            nc.vector.memset(s_sum[:], 0.0)
            nc.vector.memset(s_sq[:], 0.0)
            hpre_all = bigp.tile([fout, SLOTS], F32)

            nregs = {}
            for b in range(BLOCKS):
                cb = int(C_b[b])
                off = int(off_b[b])
                gidx_t = idxp.tile([P, cb * 8], I16, tag="gidx")
                nc.sync.dma_start(out=gidx_t[:],
                                  in_=gidx[:, off * 8:(off + cb) * 8])
                # one tile per group so the 4 queue-parallel gathers carry
                # no WAW dependency on a shared tile
                gts, gcum = [], [0]
                ccol = 0
                for g in range(GROUPS):
                    cbg = int(C_bg[b, g])
                    gcum.append(gcum[-1] + cbg)
                    if cbg == 0:
                        gts.append(None)
                        continue
                    ni = cbg * P
                    if ni not in nregs:
                        nregs[ni] = nc.gpsimd.to_reg(ni)
                    gt = gatp.tile([P, cbg * fin], BF16, tag=f"gt{g}")
                    gts.append(gt)
                    nc.gpsimd.dma_gather(
                        out_ap=gt[:].rearrange("p (c e) -> p c e", e=fin),
                        in_ap=tab[g * GROUP_ROWS:(g + 1) * GROUP_ROWS, :],
                        idxs_ap=gidx_t[:, ccol * 8:(ccol + cbg) * 8],
                        num_idxs=ni, num_idxs_reg=nregs[ni], elem_size=fin,
                        single_packet=False, queue_num=g)
                    ccol += cbg
                m = mp.tile([P, cb * P], BF16, tag="m")
                slot_b = slot_all[:, off:off + cb].rearrange(
                    "p c -> p c ()").to_broadcast([P, cb, P])
                iota_b = iota[:].rearrange("p d -> p () d").to_broadcast(
                    [P, cb, P])
                nc.vector.tensor_tensor(
                    out=m[:].rearrange("p (c d) -> p c d", d=P),
                    in0=slot_b, in1=iota_b, op=mybir.AluOpType.is_equal)
                self_t = workp.tile([P, fin], BF16, tag="self")
                nc.sync.dma_start(out=self_t[:],
                                  in_=xself[b * P:(b + 1) * P, :])
                aggT = workp.tile([P, fin], BF16, tag="aggT")
                if layer == 2:
                    # fin == P: accumulate agg TRANSPOSED directly
                    # (lhsT=gathered rows, rhs=one-hot) — skips the
                    # per-block PE transpose round-trip.
                    ps_agg = psump.tile([fin, P], F32, tag="agg",
                                        space="PSUM", bufs=2)
                    nc.tensor.matmul(out=ps_agg[:], lhsT=self_t[:],
                                     rhs=ident[:], start=True, stop=False)
                    for k in range(cb):
                        g = next(gi for gi in range(GROUPS)
                                 if gcum[gi] <= k < gcum[gi + 1])
                        kk = k - gcum[g]
                        nc.tensor.matmul(
                            out=ps_agg[:],
                            lhsT=gts[g][:, kk * fin:(kk + 1) * fin],
                            rhs=m[:, k * P:(k + 1) * P],
                            start=False, stop=(k == cb - 1))
                    nc.scalar.activation(aggT[:], ps_agg[:],
                                         mybir.ActivationFunctionType.Identity)
                else:
                    ps_agg = psump.tile([P, fin], F32, tag="agg",
                                        space="PSUM", bufs=2)
                    nc.tensor.matmul(out=ps_agg[:], lhsT=ident[:],
                                     rhs=self_t[:], start=True, stop=False)
                    for k in range(cb):
                        g = next(gi for gi in range(GROUPS)
                                 if gcum[gi] <= k < gcum[gi + 1])
                        kk = k - gcum[g]
                        nc.tensor.matmul(
                            out=ps_agg[:],
                            lhsT=m[:, k * P:(k + 1) * P],
                            rhs=gts[g][:, kk * fin:(kk + 1) * fin],
                            start=False, stop=(k == cb - 1))
                    agg_bf = workp.tile([P, fin], BF16, tag="aggbf")
                    nc.scalar.activation(agg_bf[:], ps_agg[:],
                                         mybir.ActivationFunctionType.Identity)
                    for h in range(kchunks):
                        pT = psump.tile([P, P], BF16, tag="pT", space="PSUM",
                                        bufs=2)
                        nc.tensor.transpose(out=pT[:],
                                            in_=agg_bf[:, h * P:(h + 1) * P],
                                            identity=ident[:])
                        nc.scalar.activation(
                            aggT[:, h * P:(h + 1) * P], pT[:],
                            mybir.ActivationFunctionType.Identity)
                ps1 = psump.tile([fout, P], F32, tag="ps1", space="PSUM")
                for h in range(kchunks):
                    nc.tensor.matmul(out=ps1[:], lhsT=wa_t[h][:],
                                     rhs=aggT[:, h * P:(h + 1) * P],
                                     start=(h == 0), stop=(h == kchunks - 1))
                r1 = workp.tile([fout, P], BF16, tag="r1")
                nc.scalar.activation(r1[:], ps1[:],
                                     mybir.ActivationFunctionType.Relu,
                                     bias=ba_t[:, 0:1])
                ps2 = psump.tile([fout, P], F32, tag="ps2", space="PSUM")
                nc.tensor.matmul(out=ps2[:], lhsT=wb_t[:], rhs=r1[:],
                                 start=True, stop=True)
                hs = hpre_all[:, b * P:(b + 1) * P]
                nc.scalar.activation(hs, ps2[:],
                                     mybir.ActivationFunctionType.Identity,
                                     bias=bb_t[:, 0:1])
                if b == BLOCKS - 1:
                    hstat = workp.tile([fout, P], F32, tag="hstat")
                    nc.vector.tensor_tensor(out=hstat[:], in0=hs,
                                            in1=mask[:],
                                            op=mybir.AluOpType.mult)
                    hsrc = hstat[:]
                else:
                    hsrc = hs
                part = workp.tile([fout, 1], F32, tag="part")
                nc.vector.tensor_reduce(out=part[:], in_=hsrc,
                                        axis=mybir.AxisListType.X,
                                        op=mybir.AluOpType.add)
                nc.vector.tensor_add(out=s_sum[:], in0=s_sum[:], in1=part[:])
                sq = workp.tile([fout, P], F32, tag="sq")
                nc.scalar.square(sq[:], hsrc)
                part2 = workp.tile([fout, 1], F32, tag="part2")
                nc.vector.tensor_reduce(out=part2[:], in_=sq[:],
                                        axis=mybir.AxisListType.X,
                                        op=mybir.AluOpType.add)
                nc.vector.tensor_add(out=s_sq[:], in0=s_sq[:], in1=part2[:])

            scale, shift = _emit_bn_coeffs(nc, constp, psump, dramp,
                                           s_sum, s_sq, g_t, be_t, fout,
                                           N_NODES)
            for b in range(BLOCKS):
                ht = workp.tile([fout, P], BF16, tag="ht")
                nc.scalar.activation(ht[:], hpre_all[:, b * P:(b + 1) * P],
                                     mybir.ActivationFunctionType.Relu,
                                     bias=shift[:, 0:1], scale=scale[:, 0:1])
                pT2 = psump.tile([P, fout], BF16, tag="pT2", space="PSUM", bufs=2)
                nc.tensor.transpose(out=pT2[:], in_=ht[:],
                                    identity=ident[0:fout, 0:fout])
                hrow = workp.tile([P, fout], out_dt, tag="hrow")
                nc.scalar.activation(hrow[:], pT2[:],
                                     mybir.ActivationFunctionType.Identity)
                nc.sync.dma_start(out=ho[b * P:(b + 1) * P, :], in_=hrow[:])

    lower_extended_insts(nc)
    _split_sync_waits(nc)
    return nc


# ---------------------------------------------------------------- entry
_TRACE = {"enabled": False, "exec_ns": []}


def _run(nc, in_maps):
    kw = {}
    if _TRACE["enabled"]:
        kw["trace"] = True
    res = run_bass_kernel_spmd(nc, in_maps, core_ids=list(range(N_CORES)), **kw)
    if _TRACE["enabled"]:
        _TRACE["exec_ns"].append(res.exec_time_ns)
    return res.results


def _col(v, n):
    return np.ascontiguousarray(np.asarray(v, np.float32).reshape(n, 1))


def kernel(**inputs):
    plan = _build_plan(np.asarray(inputs["edge_index"]))

    x = np.asarray(inputs["x"], np.float32)
    pos = plan["pos_of_all"]
    cores_of = np.arange(N_NODES, dtype=np.int64) // PER_CORE
    rows = (cores_of // 2) * GROUP_ROWS + (cores_of % 2) * SLOTS + pos
    xt = np.zeros((TABLE_ROWS, F_IN), dtype=bfloat16)
    xt[rows] = x.astype(bfloat16)

    w1a = np.ascontiguousarray(np.asarray(inputs["W1a"]).astype(bfloat16))
    w1b = np.ascontiguousarray(np.asarray(inputs["W1b"]).astype(bfloat16))
    w2a = np.ascontiguousarray(np.asarray(inputs["W2a"]).astype(bfloat16))
    w2b = np.ascontiguousarray(np.asarray(inputs["W2b"]).astype(bfloat16))

    xself1 = [np.zeros((SLOTS, F_IN), dtype=bfloat16) for _ in range(N_CORES)]
    for c in range(N_CORES):
        xself1[c][pos[c * PER_CORE:(c + 1) * PER_CORE]] = \
            x[c * PER_CORE:(c + 1) * PER_CORE].astype(bfloat16)

    nc1 = _build_launch(plan, 1)
    in1 = [
        {"tab": xt, "gidx": plan["cores"][c]["gidx"],
         "slotf": plan["cores"][c]["slotf"], "wa": w1a,
         "ba": _col(inputs["b1a"], H1), "wb": w1b,
         "bb": _col(inputs["b1b"], H1), "gg": _col(inputs["g1"], H1),
         "be": _col(inputs["be1"], H1), "xself": xself1[c]}
        for c in range(N_CORES)
    ]
    res1 = _run(nc1, in1)

    tab2 = np.zeros((TABLE_ROWS, H1), dtype=bfloat16)
    for c in range(N_CORES):
        r0 = (c // 2) * GROUP_ROWS + (c % 2) * SLOTS
        tab2[r0:r0 + SLOTS] = res1[c]["ho"]

    nc2 = _build_launch(plan, 2)
    in2 = [
        {"tab": tab2, "gidx": plan["cores"][c]["gidx"],
         "slotf": plan["cores"][c]["slotf"], "wa": w2a,
         "ba": _col(inputs["b2a"], H2), "wb": w2b,
         "bb": _col(inputs["b2b"], H2), "gg": _col(inputs["g2"], H2),
         "be": _col(inputs["be2"], H2), "xself": res1[c]["ho"]}
        for c in range(N_CORES)
    ]
    res2 = _run(nc2, in2)

    out = np.empty((N_NODES, H2), dtype=np.float32)
    for c in range(N_CORES):
        out[c * PER_CORE:(c + 1) * PER_CORE] = \
            res2[c]["ho"][pos[c * PER_CORE:(c + 1) * PER_CORE]]
    return out
